# revision 10
# baseline (speedup 1.0000x reference)
"""Trainium2 Bass kernel for nn_NetTGCNBasic (Chebyshev graph conv, K=25).

Strategy (8 NeuronCores, node-sharded SpMM):
  - Nodes sharded 2048/core; X_k replicated in HBM (bf16, rows padded to 512
    features = 1024 B) as the gather table.
  - Per Chebyshev step: dma_gather pulls each local edge's dst row into SBUF
    tiles [128 edges x 512]; PE "banded matmul" with host-precomputed
    stationaries stat[slot, src_local] = 2*w scatters+weights edges into PSUM
    per 64-src-node block; DVE computes X_{k+1} = PSUM - X_{k-1} (Chebyshev),
    AllGather collective refreshes the HBM replica.
  - Per step, X_k shard is PE-transposed and stored to HBM (S buffer); at the
    end one batched contraction computes out[b,n,g] = sum_k S W_k in PSUM, and
    the FC layer is a fused DVE multiply-reduce against W_fc.
  - Host does the final tiny reduction over cores/g + bias + log_softmax.
"""
import sys

sys.path.insert(0, "/opt/trn_rl_repo")

import numpy as np
import ml_dtypes

from concourse import bass, bacc, mybir, tile
from concourse import bass_utils
from concourse.masks import make_identity

# ---- problem dims (hardcoded per spec) ----
B, N, F, H, K, G, C = 32, 16384, 1, 15, 25, 64, 6
E = N * 32
NCORES = 8
NSH = N // NCORES            # 2048 nodes per core
FEAT = B * H // NCORES * NCORES  # 480 used features (b-major, h-minor)
ES = 512                     # padded row elems (1024 B rows)
NBLK = NSH // 64             # 32 src blocks of 64 nodes
TPB = 18                     # tiles (128 edge slots) per block, fixed for SPMD
SPB = TPB * 128              # 2304 slots per block
TTOT = NBLK * TPB            # 576 tiles per core
IDXC = NBLK * (SPB // 16)    # idx cols (int16, wrapped by 16)
NT = NSH // 128              # 16 node tiles per shard
QF = 4                       # feature chunks of 120 for transpose
KGROUPS = [(0, 8), (8, 16), (16, 24), (24, 25)]
NCH = 8                      # phase-B node chunks of 256
CHN = NSH // NCH             # 256

bf16 = mybir.dt.bfloat16
f32 = mybir.dt.float32
i16 = mybir.dt.int16

_CACHE = {}


def _build(num_k=K):
    nc = bacc.Bacc("TRN2", target_bir_lowering=False, debug=False,
                   num_devices=NCORES, num_swdge_queues=4)

    x0_full = nc.dram_tensor("x0_full", [N, ES], bf16, kind="ExternalInput")
    x0_sh = nc.dram_tensor("x0_sh", [128, NT, ES], bf16, kind="ExternalInput")
    idx_in = nc.dram_tensor("idx", [128, IDXC], i16, kind="ExternalInput")
    stat_in = nc.dram_tensor("stat", [128, TTOT, 64], bf16, kind="ExternalInput")
    wflat_in = nc.dram_tensor("wflat", [120, 4 * 64], bf16, kind="ExternalInput")
    wfc_in = nc.dram_tensor("wfc", [128, C, NSH], bf16, kind="ExternalInput")
    bconv_in = nc.dram_tensor("bconv", [128, 1], f32, kind="ExternalInput")
    fc_out = nc.dram_tensor("fc_parts", [128, 192], f32, kind="ExternalOutput")

    n_spmm = num_k - 1

    with tile.TileContext(nc) as tc:
        with (
            tc.tile_pool(name="persist", bufs=1) as pp,
            tc.tile_pool(name="dram", bufs=1, space="DRAM") as dram,
        ):
            # persistent SBUF residents
            idx_t = pp.tile([128, IDXC], i16)
            nc.sync.dma_start(idx_t[:], idx_in.ap())
            stat_t = pp.tile([128, TTOT, 64], bf16)
            nc.sync.dma_start(stat_t[:], stat_in.ap())
            wflat_t = pp.tile([120, 4 * 64], bf16)
            nc.sync.dma_start(wflat_t[:], wflat_in.ap())
            wfc_t = pp.tile([128, C, NSH], bf16)
            nc.sync.dma_start(wfc_t[:], wfc_in.ap())
            bconv_t = pp.tile([128, 1], f32)
            nc.sync.dma_start(bconv_t[:], bconv_in.ap())
            ident = pp.tile([128, 128], bf16)
            make_identity(nc, ident[:])

            xbuf = [pp.tile([128, NT, ES], bf16, name=f"xsh{i}") for i in range(2)]
            nc.sync.dma_start(xbuf[0][:], x0_sh.ap())

            # DRAM internals
            bounce = dram.tile([128, NT, ES], bf16)
            S = dram.tile([B, K, H, NSH], bf16)

            # ---- phase A: Chebyshev recursion ----
            with (
                tc.tile_pool(name="gpool", bufs=2) as gp,
                tc.tile_pool(name="ps_mm", bufs=4, space="PSUM") as ps_mm,
                tc.tile_pool(name="ps_tr", bufs=2, space="PSUM") as ps_tr,
                tc.tile_pool(name="strans", bufs=3) as stp,
            ):
                for k in range(num_k):
                    XC = xbuf[k % 2]
                    XP = xbuf[(k + 1) % 2]

                    # transpose-store S_k slices: S[8q+b', k, h, 128*nb+p]
                    for nb in range(NT):
                        for q in range(QF):
                            pst = ps_tr.tile([120, 128], bf16, space="PSUM")
                            nc.tensor.transpose(
                                pst[:], XC[:, nb, 120 * q:120 * (q + 1)], ident[:])
                            stg = stp.tile([120, 128], bf16)
                            nc.vector.tensor_copy(stg[:], pst[:])
                            nc.sync.dma_start(
                                S[8 * q:8 * (q + 1), k, :, 128 * nb:128 * (nb + 1)],
                                stg[:])

                    if k >= n_spmm:
                        continue

                    # SpMM: X_{k+1} = 2 L X_k - X_{k-1}
                    src_ap = x0_full.ap() if k == 0 else Rcur[:]
                    gtiles = {}
                    for blk in range(NBLK):
                        acc = ps_mm.tile([64, FEAT], f32, space="PSUM")
                        for t in range(TPB):
                            T = blk * TPB + t
                            ci = T // 8
                            if ci not in gtiles:
                                g = gp.tile([128, 8, ES], bf16,
                                            name=f"g{ci % 8}", tag="g")
                                nc.gpsimd.dma_gather(
                                    out_ap=g[:],
                                    in_ap=src_ap,
                                    idxs_ap=idx_t[:, ci * 64:(ci + 1) * 64],
                                    num_idxs=1024,
                                    num_idxs_reg=1024,
                                    elem_size=ES,
                                    queue_num=ci % 4,
                                )
                                gtiles[ci] = g
                            nc.tensor.matmul(
                                acc[:],
                                stat_t[:, T, :],
                                gtiles[ci][:, T % 8, 0:FEAT],
                                start=(t == 0),
                                stop=(t == TPB - 1),
                            )
                        p0 = 64 * (blk % 2)
                        nbf = blk // 2
                        dst = XP[p0:p0 + 64, nbf, 0:FEAT]
                        if k == 0:
                            nc.vector.tensor_scalar_mul(dst, acc[:], 0.5)
                        else:
                            nc.vector.tensor_tensor(
                                out=dst, in0=acc[:], in1=dst,
                                op=mybir.AluOpType.subtract)

                    nc.sync.dma_start(bounce[:], XP[:])
                    Rcur = dram.tile([N, ES], bf16, addr_space="Shared",
                                     name=f"R{k % 2}", tag="R", bufs=2)
                    nc.gpsimd.collective_compute(
                        "AllGather",
                        mybir.AluOpType.bypass,
                        replica_groups=[list(range(NCORES))],
                        ins=[bounce.opt()],
                        outs=[Rcur.opt()],
                    )

            # ---- phase B: contraction over (k, h) + relu + FC ----
            with (
                tc.tile_pool(name="smp", bufs=4) as smp,
                tc.tile_pool(name="ps_b", bufs=8, space="PSUM") as ps_b,
                tc.tile_pool(name="hall", bufs=1) as hp,
                tc.tile_pool(name="junk", bufs=1) as jp,
                tc.tile_pool(name="ps_fc", bufs=1, space="PSUM") as _unused_psfc,
            ):
                fc_acc = pp.tile([128, 192], f32)
                for half in range(2):
                    h_all = [hp.tile([128, NSH], bf16, name=f"h{half}_{p}")
                             for p in range(8)]
                    for ch in range(NCH):
                        pbt = [ps_b.tile([128, CHN], f32, space="PSUM",
                                         name=f"pb{_i}", tag="pb")
                               for _i in range(8)]
                        for kgi, (k0, k1) in enumerate(KGROUPS):
                            if k0 >= num_k:
                                continue
                            k1c = min(k1, num_k)
                            kd = (k1c - k0) * H
                            last_kgi = len([1 for a, _b in KGROUPS if a < num_k]) - 1
                            for bi in range(16):
                                b = half * 16 + bi
                                pair, col = bi // 2, 64 * (bi % 2)
                                sm = smp.tile([120, CHN], bf16)
                                nc.sync.dma_start(
                                    sm[0:kd, :],
                                    S[b, k0:k1c, :, ch * CHN:(ch + 1) * CHN])
                                nc.tensor.matmul(
                                    pbt[pair][col:col + 64, :],
                                    wflat_t[0:kd, 64 * kgi:64 * kgi + 64],
                                    sm[0:kd, :],
                                    start=(kgi == 0),
                                    stop=(kgi == last_kgi),
                                    tile_position=(0, col),
                                )
                        for pair in range(8):
                            # h = relu(psum + b_conv), bf16
                            nc.vector.tensor_scalar(
                                out=h_all[pair][:, ch * CHN:(ch + 1) * CHN],
                                in0=pbt[pair][:],
                                scalar1=bconv_t[:],
                                scalar2=0.0,
                                op0=mybir.AluOpType.add,
                                op1=mybir.AluOpType.max,
                            )
                    # FC: fc_acc[p, 2*((half*8+pair)*6+c)+s] = sum_n h*wfc
                    for pair in range(8):
                        for c in range(C):
                            for sub in range(2):
                                junk = jp.tile([128, NSH // 2], bf16)
                                j = 2 * ((half * 8 + pair) * 6 + c) + sub
                                lo = sub * (NSH // 2)
                                hi = lo + NSH // 2
                                nc.vector.scalar_tensor_tensor(
                                    out=junk[:],
                                    in0=h_all[pair][:, lo:hi],
                                    scalar=1.0,
                                    in1=wfc_t[:, c, lo:hi],
                                    op0=mybir.AluOpType.mult,
                                    op1=mybir.AluOpType.mult,
                                    accum_out=fc_acc[:, j:j + 1],
                                )
                nc.sync.dma_start(fc_out.ap(), fc_acc[:])

    nc.compile()
    return nc


def _row_of(n):
    """node id -> row index in the AllGather-ordered replica R."""
    return (n // NSH) * NSH + (n % 128) * NT + (n % NSH) // 128


def _prep(x, edge_weight, W, b_conv, W_fc, b_fc, edge_src, edge_dst, num_k=K):
    """Host-side input preparation. Returns in_maps (list of 8 dicts)."""
    x = np.asarray(x)
    edge_weight = np.asarray(edge_weight, np.float32)
    W = np.asarray(W, np.float32)
    b_conv = np.asarray(b_conv, np.float32)
    W_fc = np.asarray(W_fc, np.float32)
    edge_src = np.asarray(edge_src).astype(np.int64)
    edge_dst = np.asarray(edge_dst).astype(np.int64)

    X0 = np.zeros((N, ES), np.float32)
    X0[:, :FEAT] = x[:, :, 0, :].transpose(1, 0, 2).reshape(N, FEAT)
    # x0_full rows permuted so that AllGather replica layout matches:
    rows = _row_of(np.arange(N))
    x0_full = np.zeros((N, ES), ml_dtypes.bfloat16)
    x0_full[rows] = X0.astype(ml_dtypes.bfloat16)

    # gather indices address x0_full/R by permuted row id
    dst_rows = rows[edge_dst]

    wflat = np.zeros((120, 4 * 64), ml_dtypes.bfloat16)
    for kgi, (k0, k1) in enumerate(KGROUPS):
        k1c = min(k1, num_k)
        if k0 >= num_k:
            continue
        blockw = W[k0:k1c, :, 0, :]  # [kd, H, G]
        kd = (k1c - k0) * H
        wflat[0:kd, 64 * kgi:64 * kgi + 64] = (
            blockw.reshape(kd, G).astype(ml_dtypes.bfloat16))

    bconv_dup = np.tile(b_conv, 2).reshape(128, 1).astype(np.float32)

    in_maps = []
    for c in range(NCORES):
        mask = (edge_src // NSH) == c
        es = (edge_src[mask] - c * NSH).astype(np.int64)
        ed_r = dst_rows[mask]
        ew = edge_weight[mask]
        blk = es // 64
        order = np.argsort(blk, kind="stable")
        es, ed_r, ew, blk = es[order], ed_r[order], ew[order], blk[order]
        counts = np.bincount(blk, minlength=NBLK)
        assert counts.max() <= SPB, f"block overflow: {counts.max()} > {SPB}"

        slot_dst = np.zeros((NBLK, SPB), np.int64)
        slot_w2 = np.zeros((NBLK, SPB), np.float32)
        slot_src = np.zeros((NBLK, SPB), np.int64)
        pos = 0
        for bidx in range(NBLK):
            n_b = counts[bidx]
            slot_dst[bidx, :n_b] = ed_r[pos:pos + n_b]
            slot_w2[bidx, :n_b] = 2.0 * ew[pos:pos + n_b]
            slot_src[bidx, :n_b] = es[pos:pos + n_b] % 64
            pos += n_b

        # idx wrapped: [128, NBLK*144]; within block, idx j -> (16g+j%16, j//16)
        idx_w = np.zeros((128, IDXC), np.int16)
        wrap = slot_dst.reshape(NBLK, SPB // 16, 16).transpose(0, 2, 1)  # [NBLK,16,144]
        for gi in range(8):
            idx_w[gi * 16:(gi + 1) * 16, :] = (
                wrap.transpose(1, 0, 2).reshape(16, IDXC))
        # stat: [128, TTOT, 64]; slot s = 128 t + i
        stat = np.zeros((NBLK, TPB, 128, 64), np.float32)
        bb, ss = np.meshgrid(np.arange(NBLK), np.arange(SPB), indexing="ij")
        tt, ii = ss // 128, ss % 128
        stat[bb, tt, ii, slot_src] = slot_w2
        stat = (stat.reshape(NBLK * TPB, 128, 64).transpose(1, 0, 2)
                .astype(ml_dtypes.bfloat16))

        x0_sh = np.ascontiguousarray(
            X0[c * NSH:(c + 1) * NSH].reshape(NT, 128, ES).transpose(1, 0, 2)
        ).astype(ml_dtypes.bfloat16)

        # wfc: [128 (dup2,64g), C, NSH (ch*256+n')]
        ncg = (np.arange(NSH)[:, None] * G + np.arange(G)[None, :]
               + c * NSH * G)  # [n_loc, g] -> col in W_fc
        wfc = np.zeros((128, C, NSH), np.float32)
        for cc in range(C):
            m = W_fc[cc][ncg]          # [NSH, G]
            wfc[:64, cc, :] = m.T
            wfc[64:, cc, :] = m.T
        wfc = wfc.astype(ml_dtypes.bfloat16)

        in_maps.append(dict(
            x0_full=x0_full, x0_sh=x0_sh, idx=idx_w, stat=stat,
            wflat=wflat, wfc=wfc, bconv=bconv_dup,
        ))
    return in_maps


def _finish(fc_parts_all, b_fc):
    """fc_parts_all: [NCORES, 128, 96] -> log_softmax logits [B, C]."""
    logits = np.zeros((B, C), np.float64)
    for b in range(B):
        half, pair, bp = b // 16, (b % 16) // 2, b % 2
        for c in range(C):
            j = 2 * ((half * 8 + pair) * 6 + c)
            v = 0.0
            for core in range(NCORES):
                v += fc_parts_all[core][64 * bp:64 * bp + 64, j:j + 2].sum()
            logits[b, c] = v + b_fc[c]
    m = logits.max(axis=1, keepdims=True)
    ls = logits - m
    ls = ls - np.log(np.exp(ls).sum(axis=1, keepdims=True))
    return ls.astype(np.float32)


def kernel(x, edge_weight, W, b_conv, W_fc, b_fc, edge_src, edge_dst,
           _num_k=K, _trace=False):
    key = _num_k
    if key not in _CACHE:
        _CACHE[key] = _build(_num_k)
    nc = _CACHE[key]
    in_maps = _prep(x, edge_weight, W, b_conv, W_fc, b_fc,
                    edge_src, edge_dst, _num_k)
    res = bass_utils.run_bass_kernel_spmd(
        nc, in_maps, core_ids=list(range(NCORES)), trace=_trace)
    fc_parts = np.stack([r["fc_parts"].astype(np.float64) for r in res.results])
    out = _finish(fc_parts, np.asarray(b_fc, np.float64))
    if _trace:
        kernel.last_exec_time_ns = res.exec_time_ns
        kernel.last_results = res
    return out
